# revision 28
# baseline (speedup 1.0000x reference)
"""MoE kernel for trn2: 8-core data-parallel, feature-major f32r matmuls.

Math (equivalent to reference):
  sc = [ss, sl] [N,512]; vc = [vs, vl] [N,3,512]; vp = mean_v vc
  w_s = softmax(silu(sc@Wg1+bg1)@Wg2+bg2); w_v = same(vp)
  hs = silu(sc @ Ws1f + bs1f)            (experts flattened: Ws1f [512,2048])
  scalar = (hs*rep(w_s)) @ (Ws2f@Wfs) + w_s @ (bs2@Wfs) + bfs
  hv = silu(vc @ Wv1f);  vector = (hv*rep(w_v)) @ (Wv2f@Wfv)
Per core: 2500 tokens padded to 2560 = 5 tiles x 512 tokens.
"""

import numpy as np
import ml_dtypes

N, H, E = 20000, 256, 8
NCORES = 8
TPC = N // NCORES  # 2500
TPAD = 2560
TT = 512
NTILE = TPAD // TT  # 5
NB = TT // 128  # 4

_CACHE = {}


def _build():
    import concourse.tile as tile
    from concourse import bacc, mybir

    FP = mybir.dt.float32r
    F32 = mybir.dt.float32
    BF = mybir.dt.bfloat16
    AF = mybir.ActivationFunctionType

    nc = bacc.Bacc("TRN2", target_bir_lowering=False, debug=False, num_devices=NCORES)

    def din(name, shape, dt=FP):
        return nc.dram_tensor(name, shape, dt, kind="ExternalInput").ap()

    ss = din("ss", [TPAD, 256])
    sl = din("sl", [TPAD, 256])
    vs = din("vs", [TPAD, 3, 256])
    vl = din("vl", [TPAD, 3, 256])
    wg1 = din("wg1", [512, 128])
    wg2 = din("wg2", [128, 8])
    ws1 = din("ws1", [512, 2048])
    ws2 = din("ws2", [2048, 256], BF)
    wv1 = din("wv1", [512, 2048])
    wv2 = din("wv2", [2048, 256], BF)
    b2p = din("b2p", [8, 256], BF)
    sel = din("sel", [8, 1024])
    eye = din("eye", [128, 128])
    ones8 = din("ones8", [8, 1])
    consts = din("consts", [128, 20], F32)
    bg2d = din("bg2d", [8, 1], F32)
    so = nc.dram_tensor("so", [TPAD, 256], FP, kind="ExternalOutput").ap()
    vo = nc.dram_tensor("vo", [TPAD, 3, 256], FP, kind="ExternalOutput").ap()

    with tile.TileContext(nc) as tc:
        from contextlib import ExitStack

        ctx = ExitStack()
        with ctx:
            wp = ctx.enter_context(tc.tile_pool(name="wp", bufs=1))
            raw2 = ctx.enter_context(tc.tile_pool(name="raw2", bufs=4))
            raw4 = ctx.enter_context(tc.tile_pool(name="raw4", bufs=4))
            sct = ctx.enter_context(tc.tile_pool(name="sct", bufs=1))
            vtp = ctx.enter_context(tc.tile_pool(name="vtp", bufs=1))
            gsb = ctx.enter_context(tc.tile_pool(name="gsb", bufs=2))
            gsb1 = ctx.enter_context(tc.tile_pool(name="gsb1", bufs=1))
            hsb = ctx.enter_context(tc.tile_pool(name="hsb", bufs=6))
            wbp = ctx.enter_context(tc.tile_pool(name="wbp", bufs=4))
            otp = ctx.enter_context(tc.tile_pool(name="otp", bufs=2))
            outp = ctx.enter_context(tc.tile_pool(name="outp", bufs=4))
            voutp = ctx.enter_context(tc.tile_pool(name="voutp", bufs=1))
            tp = ctx.enter_context(tc.tile_pool(name="tp", bufs=2, space="PSUM"))
            hp = ctx.enter_context(tc.tile_pool(name="hp", bufs=3, space="PSUM"))
            op = ctx.enter_context(tc.tile_pool(name="op", bufs=1, space="PSUM"))
            gp = ctx.enter_context(tc.tile_pool(name="gp", bufs=1, space="PSUM"))

            # ---- weights: small ones first on the sync queue; the 4 big
            # matrices go through the gpsimd SWDGE queue so tile-0 input
            # loads (sync/HWDGE) are not head-of-line blocked behind 12MB ----
            eye_sb = wp.tile([128, 128], FP, tag="eye", name="eye")
            nc.sync.dma_start(eye_sb[:], eye[:])
            wg1_sb = wp.tile([128, 4, 128], FP, tag="wg1", name="wg1")
            nc.sync.dma_start(wg1_sb[:], wg1.rearrange("(c p) m -> p c m", p=128))
            wg2_sb = wp.tile([128, 8], FP, tag="wg2", name="wg2")
            nc.sync.dma_start(wg2_sb[:], wg2[:])
            b2p_sb = wp.tile([8, 256], BF, tag="b2p", name="b2p")
            nc.sync.dma_start(b2p_sb[:], b2p[:])
            sel_sb = wp.tile([8, 8, 128], FP, tag="sel", name="sel")
            nc.sync.dma_start(sel_sb[:], sel.rearrange("k (e m) -> k e m", m=128))
            ones8_sb = wp.tile([8, 1], FP, tag="ones8", name="ones8")
            nc.sync.dma_start(ones8_sb[:], ones8[:])
            consts_sb = wp.tile([128, 20], F32, tag="consts", name="consts")
            nc.sync.dma_start(consts_sb[:], consts[:])
            bg2_sb = wp.tile([8, 1], F32, tag="bg2", name="bg2")
            nc.sync.dma_start(bg2_sb[:], bg2d[:])
            def transpose_group(dst_slice, srcs):
                # up to 4x 128x128 transposes into one PSUM bank, one DVE copy
                n = len(srcs)
                ps = tp.tile([128, TT], FP, tag="tp", name="tps")
                for j, src in enumerate(srcs):
                    nc.tensor.transpose(ps[:, j * 128 : (j + 1) * 128], src, eye_sb[:])
                nc.vector.tensor_copy(dst_slice, ps[:, 0 : n * 128])

            def gate(xchunks, scale):
                pg = gp.tile([128, TT], F32, tag="gp", name="gp")
                for c in range(4):
                    nc.tensor.matmul(
                        pg[:], wg1_sb[:, c, :], xchunks[c][:],
                        start=(c == 0), stop=(c == 3),
                    )
                g1 = gsb.tile([128, TT], FP, tag="g1", name="g1")
                nc.scalar.activation(
                    g1[:], pg[:], AF.Silu, bias=consts_sb[:, 0:1], scale=scale
                )
                pl = gp.tile([8, TT], F32, tag="gp", name="gp")
                nc.tensor.matmul(pl[:], wg2_sb[:], g1[:], start=True, stop=True)
                eT = gsb1.tile([8, TT], FP, tag="eT", name="eT")
                nc.scalar.activation(eT[:], pl[:], AF.Exp, bias=bg2_sb[:, 0:1])
                psm = gp.tile([1, TT], F32, tag="gp", name="gp")
                nc.tensor.matmul(psm[:], ones8_sb[:], eT[:], start=True, stop=True)
                rT = gsb1.tile([1, TT], F32, tag="rT", name="rT")
                nc.vector.reciprocal(rT[:], psm[:])
                r8 = gsb1.tile([8, TT], F32, tag="r8", name="r8")
                nc.gpsimd.partition_broadcast(r8[:], rT[:])
                wT = gsb.tile([8, TT], FP, tag="wT", name="wT")
                nc.vector.tensor_mul(wT[:], eT[:], r8[:])
                wTb = gsb.tile([8, TT], BF, tag="wTb", name="wTb")
                nc.vector.tensor_copy(wTb[:], wT[:])
                return wT, wTb

            def make_wb(wT, e):
                pwb = tp.tile([128, TT], F32, tag="tp", name="tp")
                nc.tensor.matmul(
                    pwb[:], sel_sb[:, e, :], wT[:], start=True, stop=True
                )
                wb = wbp.tile([128, TT], BF, tag="wb", name="wb")
                nc.vector.tensor_copy(wb[:], pwb[:])
                return wb

            def expert_block(xchunks, w1sb, b1col, wT, wTb, w2sb, extra_b2p, bocol, outs, filler=None):
                po = [op.tile([128, TT], F32, tag=f"op{o}", name=f"op{o}") for o in range(2)]
                wb = None
                hts = [None] * 16

                def po_mms(m, last_flag):
                    for o in range(2):
                        nc.tensor.matmul(
                            po[o][:], w2sb[:, m, o * 128 : (o + 1) * 128], hts[m][:],
                            start=(m == 0), stop=last_flag,
                        )

                for m in range(16):
                    if m % 2 == 0:
                        wb = make_wb(wT, m // 2)
                    ph = hp.tile([128, TT], F32, tag="hp", name="hp")
                    for c in range(4):
                        nc.tensor.matmul(
                            ph[:], w1sb[:, c, m * 128 : (m + 1) * 128], xchunks[c][:],
                            start=(c == 0), stop=(c == 3),
                        )
                    ht = hsb.tile([128, TT], BF, tag="ht", name="ht")
                    if b1col is not None:
                        nc.scalar.activation(
                            ht[:], ph[:], AF.Silu,
                            bias=consts_sb[:, b1col + m : b1col + m + 1],
                        )
                    else:
                        nc.scalar.activation(ht[:], ph[:], AF.Silu)
                    nc.vector.tensor_mul(ht[:], ht[:], wb[:])
                    hts[m] = ht
                    if m > 2:
                        po_mms(m - 3, False)
                if filler is not None:
                    filler()
                po_mms(13, False)
                po_mms(14, False)
                po_mms(15, not extra_b2p)
                if extra_b2p:
                    for o in range(2):
                        nc.tensor.matmul(
                            po[o][:], b2p_sb[:, o * 128 : (o + 1) * 128], wTb[:],
                            start=False, stop=True,
                        )
                for o in range(2):
                    if bocol is not None:
                        nc.vector.tensor_scalar_add(
                            outs[o][:], po[o][:], consts_sb[:, bocol + o : bocol + o + 1]
                        )
                    else:
                        nc.vector.tensor_copy(outs[o][:], po[o][:])

            def load_tile(i):
                ssb, slb, vsb, vlb, vpr = [], [], [], [], []
                for b in range(NB):
                    t0 = i * TT + b * 128
                    t_s = raw2.tile([128, 256], FP, tag="ss", name="ss")
                    nc.sync.dma_start(t_s[:], ss[t0 : t0 + 128, :])
                    t_l = raw2.tile([128, 256], FP, tag="sl", name="sl")
                    nc.sync.dma_start(t_l[:], sl[t0 : t0 + 128, :])
                    t_v = raw4.tile([128, 3, 256], FP, tag="vs", name="vs")
                    nc.sync.dma_start(t_v[:], vs[t0 : t0 + 128, :, :])
                    t_w = raw4.tile([128, 3, 256], FP, tag="vl", name="vl")
                    nc.sync.dma_start(t_w[:], vl[t0 : t0 + 128, :, :])
                    ssb.append(t_s); slb.append(t_l); vsb.append(t_v); vlb.append(t_w)
                    # vp (token-major sum over channels), both halves
                    t_p = raw2.tile([128, 512], FP, tag="vpr", name="vpr")
                    nc.vector.tensor_add(t_p[:, 0:256], t_v[:, 0, :], t_v[:, 1, :])
                    nc.vector.tensor_add(t_p[:, 0:256], t_p[:, 0:256], t_v[:, 2, :])
                    nc.vector.tensor_add(t_p[:, 256:512], t_w[:, 0, :], t_w[:, 1, :])
                    nc.vector.tensor_add(t_p[:, 256:512], t_p[:, 256:512], t_w[:, 2, :])
                    vpr.append(t_p)
                return ssb, slb, vsb, vlb, vpr

            # big weight matrices AFTER tile-0 loads in program order so the
            # first tile's transposes are not blocked behind 12MB of weights
            tile0 = load_tile(0)
            ws1_sb = wp.tile([128, 4, 2048], FP, tag="ws1", name="ws1")
            nc.sync.dma_start(ws1_sb[:], ws1.rearrange("(c p) m -> p c m", p=128))
            ws2_sb = wp.tile([128, 16, 256], BF, tag="ws2", name="ws2")
            nc.sync.dma_start(ws2_sb[:], ws2.rearrange("(c p) m -> p c m", p=128))
            wv1_sb = wp.tile([128, 4, 2048], FP, tag="wv1", name="wv1")
            nc.sync.dma_start(wv1_sb[:], wv1.rearrange("(c p) m -> p c m", p=128))
            wv2_sb = wp.tile([128, 16, 256], BF, tag="wv2", name="wv2")
            nc.sync.dma_start(wv2_sb[:], wv2.rearrange("(c p) m -> p c m", p=128))

            # ================= main loop =================
            for i in range(NTILE):
                ssb, slb, vsb, vlb, vpr = tile0 if i == 0 else load_tile(i)

                scT = [sct.tile([128, TT], FP, tag=f"scT{d}", name=f"scT{d}") for d in range(4)]
                vpT = [vtp.tile([128, TT], FP, tag=f"vT{d}", name=f"vT{d}") for d in range(4)]
                for d in range(4):
                    transpose_group(
                        scT[d][:],
                        [(ssb if d < 2 else slb)[b][:, (d % 2) * 128 : (d % 2) * 128 + 128]
                         for b in range(NB)],
                    )
                # gate_s next: its cross-engine latency is hidden by the vpT
                # transposes that follow in the PE stream
                wTs, wTsb = gate(scT, 1.0)
                for d in range(4):
                    transpose_group(
                        vpT[d][:],
                        [vpr[b][:, d * 128 : (d + 1) * 128] for b in range(NB)],
                    )

                vcT0_box = {}

                def emit_vcT(v):
                    vcT = [vtp.tile([128, TT], FP, tag=f"vT{d}", name=f"vT{d}") for d in range(4)]
                    for d in range(4):
                        transpose_group(
                            vcT[d][:],
                            [(vsb if d < 2 else vlb)[b][:, v, (d % 2) * 128 : (d % 2) * 128 + 128]
                             for b in range(NB)],
                        )
                    return vcT

                # gate_v before the scalar block: its ACT/DVE tail overlaps
                # the block's dense matmul head; wTv is needed much later
                wTv, wTvb = gate(vpT, 1.0 / 3.0)
                soT = [otp.tile([128, TT], FP, tag=f"oT{o}", name=f"oT{o}") for o in range(2)]
                expert_block(scT, ws1_sb, 1, wTs, wTsb, ws2_sb, True, 17, soT,
                             filler=lambda: vcT0_box.__setitem__(0, emit_vcT(0)))
                for b in range(NB):
                    sob = outp.tile([128, 256], FP, tag="sout", name="sout")
                    transpose_group(
                        sob[:],
                        [soT[o][:, b * 128 : (b + 1) * 128] for o in range(2)],
                    )
                    t0 = i * TT + b * 128
                    nc.sync.dma_start(so[t0 : t0 + 128, :], sob[:])

                # vector path
                vob = [voutp.tile([128, 3, 256], FP, tag=f"vout{b}", name=f"vout{b}") for b in range(NB)]

                vcT_cur = vcT0_box[0]
                nxt = {}
                for v in range(3):
                    voT = [otp.tile([128, TT], FP, tag=f"oT{o}", name=f"oT{o}") for o in range(2)]
                    # next channel's transposes are emitted inside this block's
                    # pipeline tail so PE keeps dense work across the boundary
                    filler = (lambda vv=v: nxt.__setitem__(0, emit_vcT(vv + 1))) if v < 2 else None
                    expert_block(vcT_cur, wv1_sb, None, wTv, wTvb, wv2_sb, False, None, voT, filler)
                    for b in range(NB):
                        transpose_group(
                            vob[b][:, v, :],
                            [voT[o][:, b * 128 : (b + 1) * 128] for o in range(2)],
                        )
                    if v < 2:
                        vcT_cur = nxt[0]
                for b in range(NB):
                    t0 = i * TT + b * 128
                    nc.sync.dma_start(vo[t0 : t0 + 128, :, :], vob[b][:])

    nc.compile()
    return nc


def _prep_weights(Wg1, bg1, Wg2, bg2, Ws1, bs1, Ws2, bs2, Wv1, Wv2, Wfs, bfs, Wfv):
    f64 = np.float64
    ws1f = np.ascontiguousarray(Ws1.transpose(1, 0, 2).reshape(512, 2048)).astype(np.float32)
    bs1f = np.asarray(bs1).reshape(2048).astype(np.float32)
    ws2p = (np.asarray(Ws2).reshape(2048, 256).astype(f64) @ np.asarray(Wfs).astype(f64)).astype(np.float32)
    b2p = (np.asarray(bs2).astype(f64) @ np.asarray(Wfs).astype(f64)).astype(np.float32)
    wv1f = np.ascontiguousarray(Wv1.transpose(1, 0, 2).reshape(512, 2048)).astype(np.float32)
    wv2p = (np.asarray(Wv2).reshape(2048, 256).astype(f64) @ np.asarray(Wfv).astype(f64)).astype(np.float32)

    selm = np.zeros((8, 1024), np.float32)
    for e in range(8):
        selm[e, e * 128 : (e + 1) * 128] = 1.0
    consts = np.zeros((128, 20), np.float32)
    consts[:, 0] = np.asarray(bg1)
    consts[:, 1:17] = bs1f.reshape(16, 128).T
    consts[:, 17:19] = np.asarray(bfs).reshape(2, 128).T
    return dict(
        wg1=np.ascontiguousarray(Wg1).astype(np.float32),
        wg2=np.ascontiguousarray(Wg2).astype(np.float32),
        ws1=ws1f, ws2=ws2p.astype(ml_dtypes.bfloat16), wv1=wv1f,
        wv2=wv2p.astype(ml_dtypes.bfloat16), b2p=b2p.astype(ml_dtypes.bfloat16),
        sel=selm, eye=np.eye(128, dtype=np.float32),
        ones8=np.ones((8, 1), np.float32), consts=consts,
        bg2d=np.asarray(bg2).reshape(8, 1).astype(np.float32),
    )


def kernel(scalar_short, scalar_long, vector_short, vector_long,
           Wg1, bg1, Wg2, bg2, Ws1, bs1, Ws2, bs2, Wv1, Wv2, Wfs, bfs, Wfv):
    from concourse.bass_utils import run_bass_kernel_spmd

    if "nc" not in _CACHE:
        _CACHE["nc"] = _build()
    nc = _CACHE["nc"]

    wmap = _prep_weights(Wg1, bg1, Wg2, bg2, Ws1, bs1, Ws2, bs2, Wv1, Wv2, Wfs, bfs, Wfv)

    ss = np.ascontiguousarray(np.asarray(scalar_short, np.float32))
    sl = np.ascontiguousarray(np.asarray(scalar_long, np.float32))
    vs = np.ascontiguousarray(np.asarray(vector_short, np.float32))
    vl = np.ascontiguousarray(np.asarray(vector_long, np.float32))

    def shard(x, c):
        xc = x[c * TPC : (c + 1) * TPC]
        pad = [(0, TPAD - TPC)] + [(0, 0)] * (x.ndim - 1)
        return np.ascontiguousarray(np.pad(xc, pad))

    in_maps = []
    for c in range(NCORES):
        m = dict(ss=shard(ss, c), sl=shard(sl, c), vs=shard(vs, c), vl=shard(vl, c))
        m.update(wmap)
        in_maps.append(m)

    res = run_bass_kernel_spmd(nc, in_maps, core_ids=list(range(NCORES)))
    so = np.concatenate([res.results[c]["so"][:TPC] for c in range(NCORES)], 0)
    vo = np.concatenate([res.results[c]["vo"][:TPC] for c in range(NCORES)], 0)
    return so, vo


# revision 32
# speedup vs baseline: 1.2393x; 1.2393x over previous
"""MoE kernel for trn2: 8-core data-parallel, feature-major f32r matmuls.

Math (equivalent to reference):
  sc = [ss, sl] [N,512]; vc = [vs, vl] [N,3,512]; vp = mean_v vc
  w_s = softmax(silu(sc@Wg1+bg1)@Wg2+bg2); w_v = same(vp)
  hs = silu(sc @ Ws1f + bs1f)            (experts flattened: Ws1f [512,2048])
  scalar = (hs*rep(w_s)) @ (Ws2f@Wfs) + w_s @ (bs2@Wfs) + bfs
  hv = silu(vc @ Wv1f);  vector = (hv*rep(w_v)) @ (Wv2f@Wfv)
Per core: 2500 tokens padded to 2560 = 5 tiles x 512 tokens.
"""

import numpy as np
import ml_dtypes

N, H, E = 20000, 256, 8
NCORES = 8
TPC = N // NCORES  # 2500
TPAD = 2560
TT = 512
NTILE = TPAD // TT  # 5
NB = TT // 128  # 4

_CACHE = {}


def _build():
    import concourse.tile as tile
    from concourse import bacc, mybir

    FP = mybir.dt.float32r
    F32 = mybir.dt.float32
    BF = mybir.dt.bfloat16
    AF = mybir.ActivationFunctionType

    nc = bacc.Bacc("TRN2", target_bir_lowering=False, debug=False, num_devices=NCORES)

    def din(name, shape, dt=FP):
        return nc.dram_tensor(name, shape, dt, kind="ExternalInput").ap()

    ss = din("ss", [TPAD, 256])
    sl = din("sl", [TPAD, 256])
    vs = din("vs", [TPAD, 3, 256])
    vl = din("vl", [TPAD, 3, 256])
    wg1 = din("wg1", [512, 128])
    wg2 = din("wg2", [128, 8])
    ws1 = din("ws1", [512, 2048])
    ws2 = din("ws2", [2048, 256], BF)
    wv1 = din("wv1", [512, 2048])
    wv2 = din("wv2", [2048, 256], BF)
    b2p = din("b2p", [8, 256], BF)
    sel = din("sel", [8, 1024])
    eye = din("eye", [128, 128])
    ones8 = din("ones8", [8, 1])
    consts = din("consts", [128, 20], F32)
    bg2d = din("bg2d", [8, 1], F32)
    so = nc.dram_tensor("so", [TPAD, 256], FP, kind="ExternalOutput").ap()
    vo = nc.dram_tensor("vo", [TPAD, 3, 256], FP, kind="ExternalOutput").ap()

    with tile.TileContext(nc) as tc:
        from contextlib import ExitStack

        ctx = ExitStack()
        with ctx:
            wp = ctx.enter_context(tc.tile_pool(name="wp", bufs=1))
            raw2 = ctx.enter_context(tc.tile_pool(name="raw2", bufs=4))
            raw4 = ctx.enter_context(tc.tile_pool(name="raw4", bufs=4))
            sct = ctx.enter_context(tc.tile_pool(name="sct", bufs=1))
            vtp = ctx.enter_context(tc.tile_pool(name="vtp", bufs=1))
            gsb = ctx.enter_context(tc.tile_pool(name="gsb", bufs=2))
            gsb1 = ctx.enter_context(tc.tile_pool(name="gsb1", bufs=1))
            hsb = ctx.enter_context(tc.tile_pool(name="hsb", bufs=6))
            wbp = ctx.enter_context(tc.tile_pool(name="wbp", bufs=6))
            otp = ctx.enter_context(tc.tile_pool(name="otp", bufs=2))
            outp = ctx.enter_context(tc.tile_pool(name="outp", bufs=4))
            voutp = ctx.enter_context(tc.tile_pool(name="voutp", bufs=1))
            tp = ctx.enter_context(tc.tile_pool(name="tp", bufs=2, space="PSUM"))
            hp = ctx.enter_context(tc.tile_pool(name="hp", bufs=3, space="PSUM"))
            op = ctx.enter_context(tc.tile_pool(name="op", bufs=1, space="PSUM"))
            gp = ctx.enter_context(tc.tile_pool(name="gp", bufs=1, space="PSUM"))

            # ---- weights: small ones first on the sync queue; the 4 big
            # matrices go through the gpsimd SWDGE queue so tile-0 input
            # loads (sync/HWDGE) are not head-of-line blocked behind 12MB ----
            eye_sb = wp.tile([128, 128], FP, tag="eye", name="eye")
            nc.sync.dma_start(eye_sb[:], eye[:])
            wg1_sb = wp.tile([128, 4, 128], FP, tag="wg1", name="wg1")
            nc.sync.dma_start(wg1_sb[:], wg1.rearrange("(c p) m -> p c m", p=128))
            wg2_sb = wp.tile([128, 8], FP, tag="wg2", name="wg2")
            nc.sync.dma_start(wg2_sb[:], wg2[:])
            b2p_sb = wp.tile([8, 256], BF, tag="b2p", name="b2p")
            nc.sync.dma_start(b2p_sb[:], b2p[:])
            sel_sb = wp.tile([8, 8, 128], FP, tag="sel", name="sel")
            nc.sync.dma_start(sel_sb[:], sel.rearrange("k (e m) -> k e m", m=128))
            ones8_sb = wp.tile([8, 1], FP, tag="ones8", name="ones8")
            nc.sync.dma_start(ones8_sb[:], ones8[:])
            consts_sb = wp.tile([128, 20], F32, tag="consts", name="consts")
            nc.sync.dma_start(consts_sb[:], consts[:])
            bg2_sb = wp.tile([8, 1], F32, tag="bg2", name="bg2")
            nc.sync.dma_start(bg2_sb[:], bg2d[:])
            def transpose_group(dst_slice, srcs):
                # up to 4x 128x128 transposes into one PSUM bank, one DVE copy
                n = len(srcs)
                ps = tp.tile([128, TT], FP, tag="tp", name="tps")
                for j, src in enumerate(srcs):
                    nc.tensor.transpose(ps[:, j * 128 : (j + 1) * 128], src, eye_sb[:])
                nc.vector.tensor_copy(dst_slice, ps[:, 0 : n * 128])

            def gate(xchunks, scale):
                pg = gp.tile([128, TT], F32, tag="gp", name="gp")
                for c in range(4):
                    nc.tensor.matmul(
                        pg[:], wg1_sb[:, c, :], xchunks[c][:],
                        start=(c == 0), stop=(c == 3),
                    )
                g1 = gsb.tile([128, TT], FP, tag="g1", name="g1")
                nc.scalar.activation(
                    g1[:], pg[:], AF.Silu, bias=consts_sb[:, 0:1], scale=scale
                )
                pl = gp.tile([8, TT], F32, tag="gp", name="gp")
                nc.tensor.matmul(pl[:], wg2_sb[:], g1[:], start=True, stop=True)
                eT = gsb1.tile([8, TT], FP, tag="eT", name="eT")
                nc.scalar.activation(eT[:], pl[:], AF.Exp, bias=bg2_sb[:, 0:1])
                psm = gp.tile([1, TT], F32, tag="gp", name="gp")
                nc.tensor.matmul(psm[:], ones8_sb[:], eT[:], start=True, stop=True)
                rT = gsb1.tile([1, TT], F32, tag="rT", name="rT")
                nc.vector.reciprocal(rT[:], psm[:])
                r8 = gsb1.tile([8, TT], F32, tag="r8", name="r8")
                nc.gpsimd.partition_broadcast(r8[:], rT[:])
                wT = gsb.tile([8, TT], FP, tag="wT", name="wT")
                nc.vector.tensor_mul(wT[:], eT[:], r8[:])
                wTb = gsb.tile([8, TT], BF, tag="wTb", name="wTb")
                nc.vector.tensor_copy(wTb[:], wT[:])
                return wT, wTb

            def make_wb(wT, e):
                pwb = tp.tile([128, TT], F32, tag="tp", name="tp")
                nc.tensor.matmul(
                    pwb[:], sel_sb[:, e, :], wT[:], start=True, stop=True
                )
                wb = wbp.tile([128, TT], BF, tag="wb", name="wb")
                nc.vector.tensor_copy(wb[:], pwb[:])
                return wb

            def expert_block(xchunks, w1sb, b1col, wT, wTb, w2sb, extra_b2p, bocol, outs, filler=None):
                po = [op.tile([128, TT], F32, tag=f"op{o}", name=f"op{o}") for o in range(2)]
                wb = None
                hts = [None] * 16

                def po_mms(m, last_flag):
                    for o in range(2):
                        nc.tensor.matmul(
                            po[o][:], w2sb[:, m, o * 128 : (o + 1) * 128], hts[m][:],
                            start=(m == 0), stop=last_flag,
                        )

                for m in range(16):
                    if m % 2 == 0:
                        wb = make_wb(wT, m // 2)
                    ph = hp.tile([128, TT], F32, tag="hp", name="hp")
                    for c in range(4):
                        nc.tensor.matmul(
                            ph[:], w1sb[c][:, m * 128 : (m + 1) * 128], xchunks[c][:],
                            start=(c == 0), stop=(c == 3),
                        )
                    ht = hsb.tile([128, TT], BF, tag="ht", name="ht")
                    if b1col is not None:
                        nc.scalar.activation(
                            ht[:], ph[:], AF.Silu,
                            bias=consts_sb[:, b1col + m : b1col + m + 1],
                        )
                    else:
                        nc.scalar.activation(ht[:], ph[:], AF.Silu)
                    nc.vector.tensor_mul(ht[:], ht[:], wb[:])
                    hts[m] = ht
                    if m > 2:
                        po_mms(m - 3, False)
                if filler is not None:
                    filler()
                po_mms(13, False)
                po_mms(14, False)
                po_mms(15, not extra_b2p)
                if extra_b2p:
                    for o in range(2):
                        nc.tensor.matmul(
                            po[o][:], b2p_sb[:, o * 128 : (o + 1) * 128], wTb[:],
                            start=False, stop=True,
                        )
                for o in range(2):
                    if bocol is not None:
                        nc.vector.tensor_scalar_add(
                            outs[o][:], po[o][:], consts_sb[:, bocol + o : bocol + o + 1]
                        )
                    else:
                        nc.vector.tensor_copy(outs[o][:], po[o][:])

            def load_tile(i):
                ssb, slb, vsb, vlb, vpr = [], [], [], [], []
                for b in range(NB):
                    t0 = i * TT + b * 128
                    t_s = raw2.tile([128, 256], FP, tag="ss", name="ss")
                    nc.sync.dma_start(t_s[:], ss[t0 : t0 + 128, :])
                    t_l = raw2.tile([128, 256], FP, tag="sl", name="sl")
                    nc.sync.dma_start(t_l[:], sl[t0 : t0 + 128, :])
                    t_v = raw4.tile([128, 3, 256], FP, tag="vs", name="vs")
                    nc.sync.dma_start(t_v[:], vs[t0 : t0 + 128, :, :])
                    t_w = raw4.tile([128, 3, 256], FP, tag="vl", name="vl")
                    nc.sync.dma_start(t_w[:], vl[t0 : t0 + 128, :, :])
                    ssb.append(t_s); slb.append(t_l); vsb.append(t_v); vlb.append(t_w)
                    # vp (token-major sum over channels), both halves
                    t_p = raw2.tile([128, 512], FP, tag="vpr", name="vpr")
                    nc.vector.tensor_add(t_p[:, 0:256], t_v[:, 0, :], t_v[:, 1, :])
                    nc.vector.tensor_add(t_p[:, 0:256], t_p[:, 0:256], t_v[:, 2, :])
                    nc.vector.tensor_add(t_p[:, 256:512], t_w[:, 0, :], t_w[:, 1, :])
                    nc.vector.tensor_add(t_p[:, 256:512], t_p[:, 256:512], t_w[:, 2, :])
                    vpr.append(t_p)
                return ssb, slb, vsb, vlb, vpr

            # big weight matrices AFTER tile-0 loads in program order so the
            # first tile's transposes are not blocked behind 12MB of weights
            tile0 = load_tile(0)
            # per-chunk tiles: first-layer matmuls of chunk c depend only on
            # chunk c's 1MB DMA, not the whole 4MB (dep tracking is per-tile)
            ws1_r = ws1.rearrange("(c p) m -> p c m", p=128)
            ws1_cs = []
            for c_ in range(4):
                t_ = wp.tile([128, 2048], FP, tag=f"ws1c{c_}", name=f"ws1c{c_}")
                nc.sync.dma_start(t_[:], ws1_r[:, c_, :])
                ws1_cs.append(t_)
            ws2_sb = wp.tile([128, 16, 256], BF, tag="ws2", name="ws2")
            nc.sync.dma_start(ws2_sb[:], ws2.rearrange("(c p) m -> p c m", p=128))
            wv1_r = wv1.rearrange("(c p) m -> p c m", p=128)
            wv1_cs = []
            for c_ in range(4):
                t_ = wp.tile([128, 2048], FP, tag=f"wv1c{c_}", name=f"wv1c{c_}")
                nc.sync.dma_start(t_[:], wv1_r[:, c_, :])
                wv1_cs.append(t_)
            wv2_sb = wp.tile([128, 16, 256], BF, tag="wv2", name="wv2")
            nc.sync.dma_start(wv2_sb[:], wv2.rearrange("(c p) m -> p c m", p=128))

            # ================= main loop =================
            for i in range(NTILE):
                ssb, slb, vsb, vlb, vpr = tile0 if i == 0 else load_tile(i)

                scT = [sct.tile([128, TT], FP, tag=f"scT{d}", name=f"scT{d}") for d in range(4)]
                vpT = [vtp.tile([128, TT], FP, tag=f"vT{d}", name=f"vT{d}") for d in range(4)]
                for d in range(4):
                    transpose_group(
                        scT[d][:],
                        [(ssb if d < 2 else slb)[b][:, (d % 2) * 128 : (d % 2) * 128 + 128]
                         for b in range(NB)],
                    )
                # gate_s next: its cross-engine latency is hidden by the vpT
                # transposes that follow in the PE stream
                wTs, wTsb = gate(scT, 1.0)
                for d in range(4):
                    transpose_group(
                        vpT[d][:],
                        [vpr[b][:, d * 128 : (d + 1) * 128] for b in range(NB)],
                    )

                vcT0_box = {}

                def emit_vcT(v):
                    vcT = [vtp.tile([128, TT], FP, tag=f"vT{d}", name=f"vT{d}") for d in range(4)]
                    for d in range(4):
                        transpose_group(
                            vcT[d][:],
                            [(vsb if d < 2 else vlb)[b][:, v, (d % 2) * 128 : (d % 2) * 128 + 128]
                             for b in range(NB)],
                        )
                    return vcT

                # gate_v before the scalar block: its ACT/DVE tail overlaps
                # the block's dense matmul head; wTv is needed much later
                wTv, wTvb = gate(vpT, 1.0 / 3.0)
                soT = [otp.tile([128, TT], FP, tag=f"oT{o}", name=f"oT{o}") for o in range(2)]
                expert_block(scT, ws1_cs, 1, wTs, wTsb, ws2_sb, True, 17, soT,
                             filler=lambda: vcT0_box.__setitem__(0, emit_vcT(0)))
                for b in range(NB):
                    sob = outp.tile([128, 256], FP, tag="sout", name="sout")
                    transpose_group(
                        sob[:],
                        [soT[o][:, b * 128 : (b + 1) * 128] for o in range(2)],
                    )
                    t0 = i * TT + b * 128
                    nc.sync.dma_start(so[t0 : t0 + 128, :], sob[:])

                # vector path
                vob = [voutp.tile([128, 3, 256], FP, tag=f"vout{b}", name=f"vout{b}") for b in range(NB)]

                vcT_cur = vcT0_box[0]
                nxt = {}
                for v in range(3):
                    voT = [otp.tile([128, TT], FP, tag=f"oT{o}", name=f"oT{o}") for o in range(2)]
                    # next channel's transposes are emitted inside this block's
                    # pipeline tail so PE keeps dense work across the boundary
                    filler = (lambda vv=v: nxt.__setitem__(0, emit_vcT(vv + 1))) if v < 2 else None
                    expert_block(vcT_cur, wv1_cs, None, wTv, wTvb, wv2_sb, False, None, voT, filler)
                    for b in range(NB):
                        transpose_group(
                            vob[b][:, v, :],
                            [voT[o][:, b * 128 : (b + 1) * 128] for o in range(2)],
                        )
                    if v < 2:
                        vcT_cur = nxt[0]
                for b in range(NB):
                    t0 = i * TT + b * 128
                    nc.sync.dma_start(vo[t0 : t0 + 128, :, :], vob[b][:])

    nc.compile()
    return nc


def _prep_weights(Wg1, bg1, Wg2, bg2, Ws1, bs1, Ws2, bs2, Wv1, Wv2, Wfs, bfs, Wfv):
    f64 = np.float64
    ws1f = np.ascontiguousarray(Ws1.transpose(1, 0, 2).reshape(512, 2048)).astype(np.float32)
    bs1f = np.asarray(bs1).reshape(2048).astype(np.float32)
    ws2p = (np.asarray(Ws2).reshape(2048, 256).astype(f64) @ np.asarray(Wfs).astype(f64)).astype(np.float32)
    b2p = (np.asarray(bs2).astype(f64) @ np.asarray(Wfs).astype(f64)).astype(np.float32)
    wv1f = np.ascontiguousarray(Wv1.transpose(1, 0, 2).reshape(512, 2048)).astype(np.float32)
    wv2p = (np.asarray(Wv2).reshape(2048, 256).astype(f64) @ np.asarray(Wfv).astype(f64)).astype(np.float32)

    selm = np.zeros((8, 1024), np.float32)
    for e in range(8):
        selm[e, e * 128 : (e + 1) * 128] = 1.0
    consts = np.zeros((128, 20), np.float32)
    consts[:, 0] = np.asarray(bg1)
    consts[:, 1:17] = bs1f.reshape(16, 128).T
    consts[:, 17:19] = np.asarray(bfs).reshape(2, 128).T
    return dict(
        wg1=np.ascontiguousarray(Wg1).astype(np.float32),
        wg2=np.ascontiguousarray(Wg2).astype(np.float32),
        ws1=ws1f, ws2=ws2p.astype(ml_dtypes.bfloat16), wv1=wv1f,
        wv2=wv2p.astype(ml_dtypes.bfloat16), b2p=b2p.astype(ml_dtypes.bfloat16),
        sel=selm, eye=np.eye(128, dtype=np.float32),
        ones8=np.ones((8, 1), np.float32), consts=consts,
        bg2d=np.asarray(bg2).reshape(8, 1).astype(np.float32),
    )


def kernel(scalar_short, scalar_long, vector_short, vector_long,
           Wg1, bg1, Wg2, bg2, Ws1, bs1, Ws2, bs2, Wv1, Wv2, Wfs, bfs, Wfv):
    from concourse.bass_utils import run_bass_kernel_spmd

    if "nc" not in _CACHE:
        _CACHE["nc"] = _build()
    nc = _CACHE["nc"]

    wmap = _prep_weights(Wg1, bg1, Wg2, bg2, Ws1, bs1, Ws2, bs2, Wv1, Wv2, Wfs, bfs, Wfv)

    ss = np.ascontiguousarray(np.asarray(scalar_short, np.float32))
    sl = np.ascontiguousarray(np.asarray(scalar_long, np.float32))
    vs = np.ascontiguousarray(np.asarray(vector_short, np.float32))
    vl = np.ascontiguousarray(np.asarray(vector_long, np.float32))

    def shard(x, c):
        xc = x[c * TPC : (c + 1) * TPC]
        pad = [(0, TPAD - TPC)] + [(0, 0)] * (x.ndim - 1)
        return np.ascontiguousarray(np.pad(xc, pad))

    in_maps = []
    for c in range(NCORES):
        m = dict(ss=shard(ss, c), sl=shard(sl, c), vs=shard(vs, c), vl=shard(vl, c))
        m.update(wmap)
        in_maps.append(m)

    res = run_bass_kernel_spmd(nc, in_maps, core_ids=list(range(NCORES)))
    so = np.concatenate([res.results[c]["so"][:TPC] for c in range(NCORES)], 0)
    vo = np.concatenate([res.results[c]["vo"][:TPC] for c in range(NCORES)], 0)
    return so, vo
